# revision 40
# baseline (speedup 1.0000x reference)
"""AngleClassificationLoss Trainium2 kernel.

loss = BCE(probs[:,0], smooth_labels(gt_u)) + BCE(probs[:,1], smooth_labels(gt_r))

Decomposition used here (exact up to f32 rounding):
    BCE * N = -( sum(log(1-p))  +  sum_b (1/Z_b) * sum_window u*v*(log p - log(1-p)) )
where the smoothed label of example b is a separable sigma=1 gaussian centered
at (theta_bin, phi_bin), cropped to the grid and renormalized by Z_b. In f32
the gaussian tail beyond ~13 bins is below any representable contribution; a
15x15 window (+-7) changes the loss by ~1e-10 relative, far below f32 noise.

Sharding: pure data parallel over batch (1024 -> 8 cores x 128 examples).
Each core returns per-partition partial sums [128, 2]; the host reduces in f64.

Engine plan per core:
  - dense pass: 16 x [128, 8100] tiles; DMAs alternate between the SP and ACT
    HWDGE rings; one ACT Ln(1-x) per tile with accum_out per-partition sums.
    This saturates all 16 SDMA engines (~27 GB/s each) -> ~155 us, the floor.
  - bins: both channels computed 2-wide in [128, 2] ops; DVE does everything
    except Sqrt/Arctan/Exp so the pre-dense ACT stream stays ~7 us.
  - windows: 30 single-row indirect gathers (15 contiguous f32 each) overlap
    the dense pass; the tiny window Ln/reduce math is pinned behind the
    mid-stream dense Ln so it fills ACT slack without stalling the pipeline.
"""

import numpy as np

P = 128                     # examples per core (batch shard), also SBUF partitions
N_CORES = 8
N_THETA, N_PHI = 180, 360
CH = N_THETA * N_PHI        # 64800 elements per channel grid
EX = 2 * CH                 # 129600 elements per example
W = 15                      # label window size (center +/- 7)
HALF = 7
F = 5400                    # main-pass tile free size (divides EX)
NT = EX // F                # 24 tiles per core
N_MEAN = 1024 * CH          # per-channel mean divisor in the reference
RAD2BIN = 57.29577951308232  # 180/pi
PI = 3.141592653589793

_CACHE = {}


def _build_nc(dbg=False):
    import concourse.bacc as bacc
    import concourse.tile as tile
    from concourse import bass, mybir
    from concourse.tile_rust import add_dep_helper

    f32 = mybir.dt.float32
    i32 = mybir.dt.int32
    AF = mybir.ActivationFunctionType
    OP = mybir.AluOpType
    AX = mybir.AxisListType

    nc = bacc.Bacc(
        "TRN2",
        target_bir_lowering=False,
        debug=False,
        enable_asserts=False,
        num_devices=N_CORES,
    )
    probs_t = nc.dram_tensor("probs", [P, 2, N_THETA, N_PHI], f32, kind="ExternalInput")
    gt_t = [
        nc.dram_tensor("gt_u", [P, 3], f32, kind="ExternalInput"),
        nc.dram_tensor("gt_r", [P, 3], f32, kind="ExternalInput"),
    ]
    out_t = nc.dram_tensor("out", [P, 2], f32, kind="ExternalOutput")
    if dbg:
        dbg_idx = [nc.dram_tensor(f"dbg_idx{c}", [P, W], i32, kind="ExternalOutput")
                   for c in (0, 1)]
        dbg_scr = nc.dram_tensor("dbg_scr", [P, 80], f32, kind="ExternalOutput")

    probs2d = probs_t.ap().rearrange("b c t p -> b (c t p)")  # [128, 129600]
    probs1d = probs_t.ap().flatten().unsqueeze(1)             # [TOTAL, 1]

    def bcast_mid(ap2d, n):
        # [P, W] -> [P, n, W] with step-0 middle dim (free-dim broadcast)
        return bass.AP(
            tensor=ap2d.tensor,
            offset=ap2d.offset,
            ap=[list(ap2d.ap[0]), [0, n], list(ap2d.ap[1])],
        )

    with tile.TileContext(nc) as tc:
        with (
            tc.tile_pool(name="main", bufs=8) as mainp,
            tc.tile_pool(name="winp", bufs=1) as winp,
            tc.tile_pool(name="small", bufs=1) as small,
        ):
            TT = nc.vector.tensor_tensor
            TS = nc.vector.tensor_scalar

            # ---------- shared constants ----------
            jio_i = small.tile([P, W], i32)
            nc.gpsimd.iota(jio_i[:], pattern=[[1, W]], base=0, channel_multiplier=0)
            jio_f = small.tile([P, W], f32)
            nc.vector.tensor_copy(out=jio_f[:], in_=jio_i[:])
            rowio = small.tile([P, W], i32)     # r*360
            nc.gpsimd.iota(rowio[:], pattern=[[N_PHI, W]], base=0,
                           channel_multiplier=0)
            pio = small.tile([P, 1], i32)       # partition*129600
            nc.gpsimd.iota(pio[:], pattern=[[0, 1]], base=0, channel_multiplier=EX)
            choff_i = small.tile([P, 2], i32)   # [0, 1] -> scaled to [0, CH]
            nc.gpsimd.iota(choff_i[:], pattern=[[1, 2]], base=0,
                           channel_multiplier=0)

            # ---------- bins for both channels at once ([P, 2] ops) ----------
            g2 = small.tile([P, 3, 2], f32)
            for c in (0, 1):
                nc.gpsimd.dma_start(out=g2[:, :, c : c + 1],
                                    in_=gt_t[c].ap()[:, :, None])
            gx, gy, gz = g2[:, 0, :], g2[:, 1, :], g2[:, 2, :]

            scr = small.tile([P, 80], f32)
            cols = iter(range(0, 76, 2))

            def col():
                i = next(cols)
                return scr[:, i : i + 2]

            # theta = arccos(clip(z,-1,1)) via half-angle arctan:
            #   theta = (1-m)*pi + (4m-2)*arctan(sqrt(1-z^2)/(1+|z|)), m=[z>=0]
            zc = col()
            TS(out=zc, in0=gz, scalar1=1.0, scalar2=-1.0, op0=OP.min, op1=OP.max)
            z2 = col()
            TT(out=z2, in0=zc, in1=zc, op=OP.mult)
            rxy = col()
            a_sq1 = nc.scalar.activation(out=rxy, in_=z2, func=AF.Sqrt, scale=-1.0,
                                         bias=1.0)
            az = col()
            TS(out=az, in0=zc, scalar1=-1.0, scalar2=None, op0=OP.mult)
            TT(out=az, in0=az, in1=zc, op=OP.max)
            TS(out=az, in0=az, scalar1=1.0, scalar2=None, op0=OP.add)
            nc.vector.reciprocal(out=az, in_=az)
            arg = col()
            TT(out=arg, in0=rxy, in1=az, op=OP.mult)
            at = col()
            nc.scalar.activation(out=at, in_=arg, func=AF.Arctan)
            m = col()
            TS(out=m, in0=zc, scalar1=0.0, scalar2=None, op0=OP.is_ge)
            c1 = col()
            TS(out=c1, in0=m, scalar1=-PI, scalar2=PI, op0=OP.mult, op1=OP.add)
            c2 = col()
            TS(out=c2, in0=m, scalar1=4.0, scalar2=-2.0, op0=OP.mult, op1=OP.add)
            tf = col()
            TT(out=tf, in0=c2, in1=at, op=OP.mult)
            TT(out=tf, in0=tf, in1=c1, op=OP.add)
            TS(out=tf, in0=tf, scalar1=RAD2BIN, scalar2=None, op0=OP.mult)
            TS(out=tf, in0=tf, scalar1=0.0, scalar2=179.0, op0=OP.max, op1=OP.min)
            tstar_i = small.tile([P, 2], i32)
            nc.vector.tensor_copy(out=tstar_i[:], in_=tf)
            tstar = col()
            nc.vector.tensor_copy(out=tstar, in_=tstar_i[:])

            # phi = atan2(y,x) mapped to [0, 2pi):
            #   psi = (1-mx)*pi + (4mx-2)*arctan(|y|/(r+|x|)); phi = psi*(2my-1)
            #   (+2pi if negative), mx=[x>=0], my=[y>=0], r=sqrt(x^2+y^2)
            x2 = col()
            TT(out=x2, in0=gx, in1=gx, op=OP.mult)
            y2 = col()
            TT(out=y2, in0=gy, in1=gy, op=OP.mult)
            TT(out=x2, in0=x2, in1=y2, op=OP.add)
            rr = col()
            nc.scalar.activation(out=rr, in_=x2, func=AF.Sqrt)
            ax = col()
            TS(out=ax, in0=gx, scalar1=-1.0, scalar2=None, op0=OP.mult)
            TT(out=ax, in0=ax, in1=gx, op=OP.max)
            ay = col()
            TS(out=ay, in0=gy, scalar1=-1.0, scalar2=None, op0=OP.mult)
            TT(out=ay, in0=ay, in1=gy, op=OP.max)
            TT(out=ax, in0=rr, in1=ax, op=OP.add)
            TS(out=ax, in0=ax, scalar1=1e-30, scalar2=None, op0=OP.add)
            nc.vector.reciprocal(out=ax, in_=ax)
            TT(out=ay, in0=ay, in1=ax, op=OP.mult)
            a2 = col()
            a_at2 = nc.scalar.activation(out=a2, in_=ay, func=AF.Arctan)
            mx = col()
            TS(out=mx, in0=gx, scalar1=0.0, scalar2=None, op0=OP.is_ge)
            my = col()
            TS(out=my, in0=gy, scalar1=0.0, scalar2=None, op0=OP.is_ge)
            d1 = col()
            TS(out=d1, in0=mx, scalar1=4.0, scalar2=-2.0, op0=OP.mult, op1=OP.add)
            TT(out=d1, in0=d1, in1=a2, op=OP.mult)
            d2 = col()
            TS(out=d2, in0=mx, scalar1=-PI, scalar2=PI, op0=OP.mult, op1=OP.add)
            TT(out=d1, in0=d1, in1=d2, op=OP.add)   # psi = |phi|
            sy = col()
            TS(out=sy, in0=my, scalar1=2.0, scalar2=-1.0, op0=OP.mult, op1=OP.add)
            pf = col()
            TT(out=pf, in0=d1, in1=sy, op=OP.mult)
            neg = col()
            TS(out=neg, in0=pf, scalar1=0.0, scalar2=None, op0=OP.is_lt)
            TS(out=neg, in0=neg, scalar1=2.0 * PI, scalar2=None, op0=OP.mult)
            TT(out=pf, in0=pf, in1=neg, op=OP.add)
            TS(out=pf, in0=pf, scalar1=RAD2BIN, scalar2=None, op0=OP.mult)
            TS(out=pf, in0=pf, scalar1=0.0, scalar2=359.0, op0=OP.max, op1=OP.min)
            pstar_i = small.tile([P, 2], i32)
            nc.vector.tensor_copy(out=pstar_i[:], in_=pf)
            pstar = col()
            nc.vector.tensor_copy(out=pstar, in_=pstar_i[:])

            # window geometry ([P, 2])
            t0 = col()
            TS(out=t0, in0=tstar, scalar1=float(HALF), scalar2=None, op0=OP.subtract)
            TS(out=t0, in0=t0, scalar1=0.0, scalar2=float(N_THETA - W),
               op0=OP.max, op1=OP.min)
            nst = col()   # -(tstar - t0)
            TT(out=nst, in0=t0, in1=tstar, op=OP.subtract)
            p0 = col()
            TS(out=p0, in0=pstar, scalar1=float(HALF), scalar2=None, op0=OP.subtract)
            TS(out=p0, in0=p0, scalar1=0.0, scalar2=float(N_PHI - W),
               op0=OP.max, op1=OP.min)
            nsp = col()   # -(pstar - p0)
            TT(out=nsp, in0=p0, in1=pstar, op=OP.subtract)

            base = col()  # t0*360 + p0 + c*64800 (exact in f32, < 2^24)
            choff_f = col()
            nc.vector.tensor_copy(out=choff_f, in_=choff_i[:])
            TS(out=choff_f, in0=choff_f, scalar1=float(CH), scalar2=None,
               op0=OP.mult)
            TS(out=base, in0=t0, scalar1=float(N_PHI), scalar2=None, op0=OP.mult)
            TT(out=base, in0=base, in1=p0, op=OP.add)
            TT(out=base, in0=base, in1=choff_f, op=OP.add)
            base_i = small.tile([P, 2], i32)
            nc.vector.tensor_copy(out=base_i[:], in_=base)
            TT(out=base_i[:], in0=base_i[:], in1=pio[:, 0:1].to_broadcast([P, 2]),
               op=OP.add)

            # gaussian weights: d = jio - shift; w = exp(-0.5*d^2)
            # (f32 underflow of the tails implements the mask crop)
            wts = {}
            exps = []
            for c in (0, 1):
                vv = small.tile([P, W], f32, tag=f"vv{c}")
                TS(out=vv[:], in0=jio_f[:], scalar1=nsp[:, c : c + 1], scalar2=None,
                   op0=OP.add)
                TT(out=vv[:], in0=vv[:], in1=vv[:], op=OP.mult)
                exps.append(nc.scalar.activation(out=vv[:], in_=vv[:], func=AF.Exp,
                                                 scale=-0.5))
                uu = small.tile([P, W], f32, tag=f"uu{c}")
                TS(out=uu[:], in0=jio_f[:], scalar1=nst[:, c : c + 1], scalar2=None,
                   op0=OP.add)
                TT(out=uu[:], in0=uu[:], in1=uu[:], op=OP.mult)
                exps.append(nc.scalar.activation(out=uu[:], in_=uu[:], func=AF.Exp,
                                                 scale=-0.5))
                zz = col()
                nc.vector.tensor_reduce(out=zz[:, 0:1], in_=uu[:], axis=AX.X,
                                        op=OP.add)
                nc.vector.tensor_reduce(out=zz[:, 1:2], in_=vv[:], axis=AX.X,
                                        op=OP.add)
                rz = scr[:, 76 + c : 77 + c]
                TT(out=rz, in0=zz[:, 0:1], in1=zz[:, 1:2], op=OP.mult)
                nc.vector.reciprocal(out=rz, in_=rz)
                wts[c] = (vv, uu, rz)

            # ---------- window gathers (overlap the dense pass) ----------
            ch = {}
            for c in (0, 1):
                idx = small.tile([P, W], i32, tag=f"idx{c}")
                TT(out=idx[:], in0=rowio[:],
                   in1=base_i[:, c : c + 1].to_broadcast([P, W]), op=OP.add)
                win = winp.tile([P, W, W], f32, tag=f"win{c}")
                for r in range(W):
                    nc.gpsimd.indirect_dma_start(
                        out=win[:, r, :],
                        out_offset=None,
                        in_=probs1d,
                        in_offset=bass.IndirectOffsetOnAxis(
                            ap=idx[:, r : r + 1], axis=0
                        ),
                    )
                ch[c] = dict(win=win, idx=idx, s2c=col())
                if dbg:
                    nc.sync.dma_start(out=dbg_idx[c].ap(), in_=idx[:])

            s2tot = small.tile([P, 1], f32)
            nc.vector.memset(s2tot[:], 0.0)

            # ---------- dense pass: sum log(1-p) over everything ----------
            # DMAs alternate between the two HWDGE rings (SP and ACT) so one
            # ring's completion latency hides under the other's transfer.
            stats = small.tile([P, NT], f32)
            main_lns = []
            for i in range(NT):
                mt = mainp.tile([P, F], f32, tag="mt")
                eng = nc.sync if i % 2 == 0 else nc.scalar
                eng.dma_start(out=mt[:], in_=probs2d[:, i * F : (i + 1) * F])
                ln = nc.scalar.activation(
                    out=mt[:], in_=mt[:], func=AF.Ln, scale=-1.0,
                    bias=1.0, accum_out=stats[:, i : i + 1])
                main_lns.append(ln)
            pin_main = main_lns[NT * 5 // 8]
            # pin the weight Exps mid-stream too (their own table loads would
            # otherwise interleave with the early dense Lns)
            for a in exps:
                add_dep_helper(a.ins, pin_main.ins, sync=False,
                               reason="weight Exps after mid dense Ln")

            # ---------- window math (pinned behind the mid dense Ln) ----------
            # The tile cost model underestimates SWDGE gather latency; by
            # mid-stream the gathers are done and the remaining dense Lns hide
            # any residual wait.
            for c in (0, 1):
                win = ch[c]["win"]
                vv, uu, rz = wts[c]
                win2 = win[:].rearrange("p a b -> p (a b)")
                lq = winp.tile([P, W * W], f32, tag=f"lq{c}")
                w1 = nc.scalar.activation(out=lq[:], in_=win2, func=AF.Ln,
                                          scale=-1.0, bias=1.0)
                w2 = nc.scalar.activation(out=win2, in_=win2, func=AF.Ln)
                for w in (w1, w2):
                    add_dep_helper(w.ins, pin_main.ins, sync=False,
                                   reason="window Lns after mid dense Ln on ACT")
                TT(out=lq[:], in0=win2, in1=lq[:], op=OP.subtract)
                lq3 = lq[:].rearrange("p (a b) -> p a b", a=W)
                TT(out=lq3, in0=lq3, in1=bcast_mid(vv[:], W), op=OP.mult)
                rsum = small.tile([P, W], f32, tag=f"rs{c}")
                nc.vector.tensor_reduce(out=rsum[:], in_=lq3, axis=AX.X, op=OP.add)
                TT(out=rsum[:], in0=rsum[:], in1=uu[:], op=OP.mult)
                s2c = ch[c]["s2c"]
                nc.vector.tensor_reduce(out=s2c[:, 0:1], in_=rsum[:], axis=AX.X,
                                        op=OP.add)
                TT(out=s2c[:, 0:1], in0=s2c[:, 0:1], in1=rz, op=OP.mult)
                TT(out=s2tot[:], in0=s2tot[:], in1=s2c[:, 0:1], op=OP.add)

            # ---------- assemble output ----------
            outt = small.tile([P, 2], f32)
            nc.vector.tensor_reduce(out=outt[:, 0:1], in_=stats[:], axis=AX.X,
                                    op=OP.add)
            nc.vector.tensor_copy(out=outt[:, 1:2], in_=s2tot[:])
            nc.sync.dma_start(out=out_t.ap(), in_=outt[:])
            if dbg:
                nc.sync.dma_start(out=dbg_scr.ap(), in_=scr[:])

    nc.compile()
    return nc


def _get_nc():
    if "nc" not in _CACHE:
        _CACHE["nc"] = _build_nc()
    return _CACHE["nc"]


def _run_on_hw(in_maps, trace=False, **kw):
    from concourse.bass_utils import run_bass_kernel_spmd

    return run_bass_kernel_spmd(_get_nc(), in_maps, core_ids=list(range(N_CORES)),
                                trace=trace, **kw)


def _make_in_maps(probs, gt_u, gt_r):
    probs = np.ascontiguousarray(np.asarray(probs, dtype=np.float32))
    gt_u = np.ascontiguousarray(np.asarray(gt_u, dtype=np.float32))
    gt_r = np.ascontiguousarray(np.asarray(gt_r, dtype=np.float32))
    assert probs.shape == (N_CORES * P, 2, N_THETA, N_PHI)
    return [
        {
            "probs": probs[i * P : (i + 1) * P],
            "gt_u": gt_u[i * P : (i + 1) * P],
            "gt_r": gt_r[i * P : (i + 1) * P],
        }
        for i in range(N_CORES)
    ]


def _combine(results):
    tot = np.float64(0.0)
    for r in results:
        tot += r["out"].astype(np.float64).sum()
    return np.float32(-(tot / N_MEAN))


def kernel(probs, gt_u, gt_r):
    res = _run_on_hw(_make_in_maps(probs, gt_u, gt_r))
    return _combine(res.results)


# revision 41
# speedup vs baseline: 1.0177x; 1.0177x over previous
"""AngleClassificationLoss Trainium2 kernel.

loss = BCE(probs[:,0], smooth_labels(gt_u)) + BCE(probs[:,1], smooth_labels(gt_r))

Decomposition used here (exact up to f32 rounding):
    BCE * N = -( sum(log(1-p))  +  sum_b (1/Z_b) * sum_window u*v*(log p - log(1-p)) )
where the smoothed label of example b is a separable sigma=1 gaussian centered
at (theta_bin, phi_bin), cropped to the grid and renormalized by Z_b. In f32
the gaussian tail beyond ~13 bins is below any representable contribution; a
15x15 window (+-7) changes the loss by ~1e-10 relative, far below f32 noise.

Sharding: pure data parallel over batch (1024 -> 8 cores x 128 examples).
Each core returns per-partition partial sums [128, 2]; the host reduces in f64.

Engine plan per core:
  - dense pass: 20 x [128, 6480] tiles; DMAs alternate between the SP and ACT
    HWDGE rings; one ACT Ln(1-x) per tile with accum_out per-partition sums.
    This saturates all 16 SDMA engines (~27 GB/s each) -> ~155 us, the floor.
    bufs=7 gives enough slack that the late-starting Ln stream never starves
    the DMA ring at the end of the pass.
  - bins: both channels computed 2-wide in [128, 2] ops; DVE does everything
    except Sqrt/Arctan/Exp so the pre-dense ACT stream stays ~7 us.
  - windows: 30 single-row indirect gathers (15 contiguous f32 each) overlap
    the dense pass; the tiny window Ln/reduce math is pinned behind the
    mid-stream dense Ln so it fills ACT slack without stalling the pipeline.
"""

import numpy as np

P = 128                     # examples per core (batch shard), also SBUF partitions
N_CORES = 8
N_THETA, N_PHI = 180, 360
CH = N_THETA * N_PHI        # 64800 elements per channel grid
EX = 2 * CH                 # 129600 elements per example
W = 15                      # label window size (center +/- 7)
HALF = 7
F = 6480                    # main-pass tile free size (divides EX)
NT = EX // F                # 20 tiles per core
N_MEAN = 1024 * CH          # per-channel mean divisor in the reference
RAD2BIN = 57.29577951308232  # 180/pi
PI = 3.141592653589793

_CACHE = {}


def _build_nc(dbg=False):
    import concourse.bacc as bacc
    import concourse.tile as tile
    from concourse import bass, mybir
    from concourse.tile_rust import add_dep_helper

    f32 = mybir.dt.float32
    i32 = mybir.dt.int32
    AF = mybir.ActivationFunctionType
    OP = mybir.AluOpType
    AX = mybir.AxisListType

    nc = bacc.Bacc(
        "TRN2",
        target_bir_lowering=False,
        debug=False,
        enable_asserts=False,
        num_devices=N_CORES,
    )
    probs_t = nc.dram_tensor("probs", [P, 2, N_THETA, N_PHI], f32, kind="ExternalInput")
    gt_t = [
        nc.dram_tensor("gt_u", [P, 3], f32, kind="ExternalInput"),
        nc.dram_tensor("gt_r", [P, 3], f32, kind="ExternalInput"),
    ]
    out_t = nc.dram_tensor("out", [P, 2], f32, kind="ExternalOutput")
    if dbg:
        dbg_idx = [nc.dram_tensor(f"dbg_idx{c}", [P, W], i32, kind="ExternalOutput")
                   for c in (0, 1)]
        dbg_scr = nc.dram_tensor("dbg_scr", [P, 80], f32, kind="ExternalOutput")

    probs2d = probs_t.ap().rearrange("b c t p -> b (c t p)")  # [128, 129600]
    probs1d = probs_t.ap().flatten().unsqueeze(1)             # [TOTAL, 1]

    def bcast_mid(ap2d, n):
        # [P, W] -> [P, n, W] with step-0 middle dim (free-dim broadcast)
        return bass.AP(
            tensor=ap2d.tensor,
            offset=ap2d.offset,
            ap=[list(ap2d.ap[0]), [0, n], list(ap2d.ap[1])],
        )

    with tile.TileContext(nc) as tc:
        with (
            tc.tile_pool(name="main", bufs=7) as mainp,
            tc.tile_pool(name="winp", bufs=1) as winp,
            tc.tile_pool(name="small", bufs=1) as small,
        ):
            TT = nc.vector.tensor_tensor
            TS = nc.vector.tensor_scalar

            # ---------- shared constants ----------
            jio_i = small.tile([P, W], i32)
            nc.gpsimd.iota(jio_i[:], pattern=[[1, W]], base=0, channel_multiplier=0)
            jio_f = small.tile([P, W], f32)
            nc.vector.tensor_copy(out=jio_f[:], in_=jio_i[:])
            rowio = small.tile([P, W], i32)     # r*360
            nc.gpsimd.iota(rowio[:], pattern=[[N_PHI, W]], base=0,
                           channel_multiplier=0)
            pio = small.tile([P, 1], i32)       # partition*129600
            nc.gpsimd.iota(pio[:], pattern=[[0, 1]], base=0, channel_multiplier=EX)
            choff_i = small.tile([P, 2], i32)   # [0, 1] -> scaled to [0, CH]
            nc.gpsimd.iota(choff_i[:], pattern=[[1, 2]], base=0,
                           channel_multiplier=0)

            # ---------- bins for both channels at once ([P, 2] ops) ----------
            g2 = small.tile([P, 3, 2], f32)
            for c in (0, 1):
                nc.gpsimd.dma_start(out=g2[:, :, c : c + 1],
                                    in_=gt_t[c].ap()[:, :, None])
            gx, gy, gz = g2[:, 0, :], g2[:, 1, :], g2[:, 2, :]

            scr = small.tile([P, 80], f32)
            cols = iter(range(0, 76, 2))

            def col():
                i = next(cols)
                return scr[:, i : i + 2]

            # theta = arccos(clip(z,-1,1)) via half-angle arctan:
            #   theta = (1-m)*pi + (4m-2)*arctan(sqrt(1-z^2)/(1+|z|)), m=[z>=0]
            zc = col()
            TS(out=zc, in0=gz, scalar1=1.0, scalar2=-1.0, op0=OP.min, op1=OP.max)
            z2 = col()
            TT(out=z2, in0=zc, in1=zc, op=OP.mult)
            rxy = col()
            a_sq1 = nc.scalar.activation(out=rxy, in_=z2, func=AF.Sqrt, scale=-1.0,
                                         bias=1.0)
            az = col()
            TS(out=az, in0=zc, scalar1=-1.0, scalar2=None, op0=OP.mult)
            TT(out=az, in0=az, in1=zc, op=OP.max)
            TS(out=az, in0=az, scalar1=1.0, scalar2=None, op0=OP.add)
            nc.vector.reciprocal(out=az, in_=az)
            arg = col()
            TT(out=arg, in0=rxy, in1=az, op=OP.mult)
            at = col()
            nc.scalar.activation(out=at, in_=arg, func=AF.Arctan)
            m = col()
            TS(out=m, in0=zc, scalar1=0.0, scalar2=None, op0=OP.is_ge)
            c1 = col()
            TS(out=c1, in0=m, scalar1=-PI, scalar2=PI, op0=OP.mult, op1=OP.add)
            c2 = col()
            TS(out=c2, in0=m, scalar1=4.0, scalar2=-2.0, op0=OP.mult, op1=OP.add)
            tf = col()
            TT(out=tf, in0=c2, in1=at, op=OP.mult)
            TT(out=tf, in0=tf, in1=c1, op=OP.add)
            TS(out=tf, in0=tf, scalar1=RAD2BIN, scalar2=None, op0=OP.mult)
            TS(out=tf, in0=tf, scalar1=0.0, scalar2=179.0, op0=OP.max, op1=OP.min)
            tstar_i = small.tile([P, 2], i32)
            nc.vector.tensor_copy(out=tstar_i[:], in_=tf)
            tstar = col()
            nc.vector.tensor_copy(out=tstar, in_=tstar_i[:])

            # phi = atan2(y,x) mapped to [0, 2pi):
            #   psi = (1-mx)*pi + (4mx-2)*arctan(|y|/(r+|x|)); phi = psi*(2my-1)
            #   (+2pi if negative), mx=[x>=0], my=[y>=0], r=sqrt(x^2+y^2)
            x2 = col()
            TT(out=x2, in0=gx, in1=gx, op=OP.mult)
            y2 = col()
            TT(out=y2, in0=gy, in1=gy, op=OP.mult)
            TT(out=x2, in0=x2, in1=y2, op=OP.add)
            rr = col()
            nc.scalar.activation(out=rr, in_=x2, func=AF.Sqrt)
            ax = col()
            TS(out=ax, in0=gx, scalar1=-1.0, scalar2=None, op0=OP.mult)
            TT(out=ax, in0=ax, in1=gx, op=OP.max)
            ay = col()
            TS(out=ay, in0=gy, scalar1=-1.0, scalar2=None, op0=OP.mult)
            TT(out=ay, in0=ay, in1=gy, op=OP.max)
            TT(out=ax, in0=rr, in1=ax, op=OP.add)
            TS(out=ax, in0=ax, scalar1=1e-30, scalar2=None, op0=OP.add)
            nc.vector.reciprocal(out=ax, in_=ax)
            TT(out=ay, in0=ay, in1=ax, op=OP.mult)
            a2 = col()
            a_at2 = nc.scalar.activation(out=a2, in_=ay, func=AF.Arctan)
            mx = col()
            TS(out=mx, in0=gx, scalar1=0.0, scalar2=None, op0=OP.is_ge)
            my = col()
            TS(out=my, in0=gy, scalar1=0.0, scalar2=None, op0=OP.is_ge)
            d1 = col()
            TS(out=d1, in0=mx, scalar1=4.0, scalar2=-2.0, op0=OP.mult, op1=OP.add)
            TT(out=d1, in0=d1, in1=a2, op=OP.mult)
            d2 = col()
            TS(out=d2, in0=mx, scalar1=-PI, scalar2=PI, op0=OP.mult, op1=OP.add)
            TT(out=d1, in0=d1, in1=d2, op=OP.add)   # psi = |phi|
            sy = col()
            TS(out=sy, in0=my, scalar1=2.0, scalar2=-1.0, op0=OP.mult, op1=OP.add)
            pf = col()
            TT(out=pf, in0=d1, in1=sy, op=OP.mult)
            neg = col()
            TS(out=neg, in0=pf, scalar1=0.0, scalar2=None, op0=OP.is_lt)
            TS(out=neg, in0=neg, scalar1=2.0 * PI, scalar2=None, op0=OP.mult)
            TT(out=pf, in0=pf, in1=neg, op=OP.add)
            TS(out=pf, in0=pf, scalar1=RAD2BIN, scalar2=None, op0=OP.mult)
            TS(out=pf, in0=pf, scalar1=0.0, scalar2=359.0, op0=OP.max, op1=OP.min)
            pstar_i = small.tile([P, 2], i32)
            nc.vector.tensor_copy(out=pstar_i[:], in_=pf)
            pstar = col()
            nc.vector.tensor_copy(out=pstar, in_=pstar_i[:])

            # window geometry ([P, 2])
            t0 = col()
            TS(out=t0, in0=tstar, scalar1=float(HALF), scalar2=None, op0=OP.subtract)
            TS(out=t0, in0=t0, scalar1=0.0, scalar2=float(N_THETA - W),
               op0=OP.max, op1=OP.min)
            nst = col()   # -(tstar - t0)
            TT(out=nst, in0=t0, in1=tstar, op=OP.subtract)
            p0 = col()
            TS(out=p0, in0=pstar, scalar1=float(HALF), scalar2=None, op0=OP.subtract)
            TS(out=p0, in0=p0, scalar1=0.0, scalar2=float(N_PHI - W),
               op0=OP.max, op1=OP.min)
            nsp = col()   # -(pstar - p0)
            TT(out=nsp, in0=p0, in1=pstar, op=OP.subtract)

            base = col()  # t0*360 + p0 + c*64800 (exact in f32, < 2^24)
            choff_f = col()
            nc.vector.tensor_copy(out=choff_f, in_=choff_i[:])
            TS(out=choff_f, in0=choff_f, scalar1=float(CH), scalar2=None,
               op0=OP.mult)
            TS(out=base, in0=t0, scalar1=float(N_PHI), scalar2=None, op0=OP.mult)
            TT(out=base, in0=base, in1=p0, op=OP.add)
            TT(out=base, in0=base, in1=choff_f, op=OP.add)
            base_i = small.tile([P, 2], i32)
            nc.vector.tensor_copy(out=base_i[:], in_=base)
            TT(out=base_i[:], in0=base_i[:], in1=pio[:, 0:1].to_broadcast([P, 2]),
               op=OP.add)

            # gaussian weights: d = jio - shift; w = exp(-0.5*d^2)
            # (f32 underflow of the tails implements the mask crop)
            wts = {}
            exps = []
            for c in (0, 1):
                vv = small.tile([P, W], f32, tag=f"vv{c}")
                TS(out=vv[:], in0=jio_f[:], scalar1=nsp[:, c : c + 1], scalar2=None,
                   op0=OP.add)
                TT(out=vv[:], in0=vv[:], in1=vv[:], op=OP.mult)
                exps.append(nc.scalar.activation(out=vv[:], in_=vv[:], func=AF.Exp,
                                                 scale=-0.5))
                uu = small.tile([P, W], f32, tag=f"uu{c}")
                TS(out=uu[:], in0=jio_f[:], scalar1=nst[:, c : c + 1], scalar2=None,
                   op0=OP.add)
                TT(out=uu[:], in0=uu[:], in1=uu[:], op=OP.mult)
                exps.append(nc.scalar.activation(out=uu[:], in_=uu[:], func=AF.Exp,
                                                 scale=-0.5))
                zz = col()
                nc.vector.tensor_reduce(out=zz[:, 0:1], in_=uu[:], axis=AX.X,
                                        op=OP.add)
                nc.vector.tensor_reduce(out=zz[:, 1:2], in_=vv[:], axis=AX.X,
                                        op=OP.add)
                rz = scr[:, 76 + c : 77 + c]
                TT(out=rz, in0=zz[:, 0:1], in1=zz[:, 1:2], op=OP.mult)
                nc.vector.reciprocal(out=rz, in_=rz)
                wts[c] = (vv, uu, rz)

            # ---------- window gathers (overlap the dense pass) ----------
            ch = {}
            for c in (0, 1):
                idx = small.tile([P, W], i32, tag=f"idx{c}")
                TT(out=idx[:], in0=rowio[:],
                   in1=base_i[:, c : c + 1].to_broadcast([P, W]), op=OP.add)
                win = winp.tile([P, W, W], f32, tag=f"win{c}")
                for r in range(W):
                    nc.gpsimd.indirect_dma_start(
                        out=win[:, r, :],
                        out_offset=None,
                        in_=probs1d,
                        in_offset=bass.IndirectOffsetOnAxis(
                            ap=idx[:, r : r + 1], axis=0
                        ),
                    )
                ch[c] = dict(win=win, idx=idx, s2c=col())
                if dbg:
                    nc.sync.dma_start(out=dbg_idx[c].ap(), in_=idx[:])

            s2tot = small.tile([P, 1], f32)
            nc.vector.memset(s2tot[:], 0.0)

            # ---------- dense pass: sum log(1-p) over everything ----------
            # DMAs alternate between the two HWDGE rings (SP and ACT) so one
            # ring's completion latency hides under the other's transfer.
            stats = small.tile([P, NT], f32)
            main_lns = []
            for i in range(NT):
                mt = mainp.tile([P, F], f32, tag="mt")
                eng = nc.sync if i % 2 == 0 else nc.scalar
                eng.dma_start(out=mt[:], in_=probs2d[:, i * F : (i + 1) * F])
                ln = nc.scalar.activation(
                    out=mt[:], in_=mt[:], func=AF.Ln, scale=-1.0,
                    bias=1.0, accum_out=stats[:, i : i + 1])
                main_lns.append(ln)
            pin_main = main_lns[NT * 5 // 8]
            # pin the weight Exps mid-stream too (their own table loads would
            # otherwise interleave with the early dense Lns)
            for a in exps:
                add_dep_helper(a.ins, pin_main.ins, sync=False,
                               reason="weight Exps after mid dense Ln")

            # ---------- window math (pinned behind the mid dense Ln) ----------
            # The tile cost model underestimates SWDGE gather latency; by
            # mid-stream the gathers are done and the remaining dense Lns hide
            # any residual wait.
            for c in (0, 1):
                win = ch[c]["win"]
                vv, uu, rz = wts[c]
                win2 = win[:].rearrange("p a b -> p (a b)")
                lq = winp.tile([P, W * W], f32, tag=f"lq{c}")
                w1 = nc.scalar.activation(out=lq[:], in_=win2, func=AF.Ln,
                                          scale=-1.0, bias=1.0)
                w2 = nc.scalar.activation(out=win2, in_=win2, func=AF.Ln)
                for w in (w1, w2):
                    add_dep_helper(w.ins, pin_main.ins, sync=False,
                                   reason="window Lns after mid dense Ln on ACT")
                TT(out=lq[:], in0=win2, in1=lq[:], op=OP.subtract)
                lq3 = lq[:].rearrange("p (a b) -> p a b", a=W)
                TT(out=lq3, in0=lq3, in1=bcast_mid(vv[:], W), op=OP.mult)
                rsum = small.tile([P, W], f32, tag=f"rs{c}")
                nc.vector.tensor_reduce(out=rsum[:], in_=lq3, axis=AX.X, op=OP.add)
                TT(out=rsum[:], in0=rsum[:], in1=uu[:], op=OP.mult)
                s2c = ch[c]["s2c"]
                nc.vector.tensor_reduce(out=s2c[:, 0:1], in_=rsum[:], axis=AX.X,
                                        op=OP.add)
                TT(out=s2c[:, 0:1], in0=s2c[:, 0:1], in1=rz, op=OP.mult)
                TT(out=s2tot[:], in0=s2tot[:], in1=s2c[:, 0:1], op=OP.add)

            # ---------- assemble output ----------
            outt = small.tile([P, 2], f32)
            nc.vector.tensor_reduce(out=outt[:, 0:1], in_=stats[:], axis=AX.X,
                                    op=OP.add)
            nc.vector.tensor_copy(out=outt[:, 1:2], in_=s2tot[:])
            nc.sync.dma_start(out=out_t.ap(), in_=outt[:])
            if dbg:
                nc.sync.dma_start(out=dbg_scr.ap(), in_=scr[:])

    nc.compile()
    return nc


def _get_nc():
    if "nc" not in _CACHE:
        _CACHE["nc"] = _build_nc()
    return _CACHE["nc"]


def _run_on_hw(in_maps, trace=False, **kw):
    from concourse.bass_utils import run_bass_kernel_spmd

    return run_bass_kernel_spmd(_get_nc(), in_maps, core_ids=list(range(N_CORES)),
                                trace=trace, **kw)


def _make_in_maps(probs, gt_u, gt_r):
    probs = np.ascontiguousarray(np.asarray(probs, dtype=np.float32))
    gt_u = np.ascontiguousarray(np.asarray(gt_u, dtype=np.float32))
    gt_r = np.ascontiguousarray(np.asarray(gt_r, dtype=np.float32))
    assert probs.shape == (N_CORES * P, 2, N_THETA, N_PHI)
    return [
        {
            "probs": probs[i * P : (i + 1) * P],
            "gt_u": gt_u[i * P : (i + 1) * P],
            "gt_r": gt_r[i * P : (i + 1) * P],
        }
        for i in range(N_CORES)
    ]


def _combine(results):
    tot = np.float64(0.0)
    for r in results:
        tot += r["out"].astype(np.float64).sum()
    return np.float32(-(tot / N_MEAN))


def kernel(probs, gt_u, gt_r):
    res = _run_on_hw(_make_in_maps(probs, gt_u, gt_r))
    return _combine(res.results)
